# revision 25
# baseline (speedup 1.0000x reference)
"""Causal flash attention (B=2, H=16, S=2048, D=64, fp32) on 8 TRN2 NeuronCores.

Strategy: shard batch*heads (32) across 8 cores -> 4 heads/core, processed as
two packed head-pairs (2 heads x 64 d on the 128 SBUF partitions). Per head,
transposed scores S^T[k, q] = K Q^T via PE (fp16, softmax scale pre-folded
into k on the host), exp split across TWO engines to break the ACT wall:

 - ACT (scalar) engine: exact spline exp for most k-tile chunks.
 - DVE (vector) engine: Schraudolph bit-trick exp for the diagonal chunks
   (plus a few chunks for load balance): i32 = round(s*A + M);
   bitcast(i32) ~= exp(s). The additive constant M doubles as the causal
   mask: valid lanes get the Schraudolph bias B, garbage lanes get +5e8
   whose bitcast is ~1e-26, i.e. p = 0.

q-block 0 of each head stays fully on ACT (exact) with a multiplicative
0/1 fp16 mask so the shortest softmax rows are never approximated. DVE
chunks are interleaved mid-q-block so neither engine bubbles.

PSUM collision granularity on TRN2 is a 4KB even-odd bank PAIR (a 3-bank
score tile straddling a pair boundary while the PE writes the neighbour
hard-crashes the device). Layout: three 2-bank score buffers in pairs
{0,1} {2,3} {4,5}; PV accumulators (both heads packed, [65, 512]) in banks
6 and 7. PV via PE with a ones column appended to V so the denominator
falls out of the same matmul. Output leaves transposed ([d+1, q] fp32);
host divides and transposes back.
"""

import os

import numpy as np

B, H, S, D = 2, 16, 2048, 64
BH = B * H
NCORES = 8
HPC = BH // NCORES  # heads per core
SCALE = 0.125
W = 256             # q-block width
TK = 128            # k-tile height
NKT = S // TK       # 16 k-tiles
NQB = S // W        # 8 q-blocks
G = 2               # k-tiles per score group: [128, 2*G*W] fp32 = 1 bank pair

A_SCH = 12102203.161561485   # 2**23 / ln(2)
B_SCH = 1064866805.0         # 127 * 2**23 - 486411 (balanced Schraudolph bias)
GARB = 5.0e8                 # garbage lanes: bitcast(i32(~5e8)) ~ 1e-16 -> p=0

# q-blocks (per pair) whose pre-diagonal chunk also goes to the DVE path
# (engine load balancing; tiles are exact on ACT, ~3%-approx on DVE).
EXTRA_QBS = (5, 6, 7)

_PSUM2 = os.environ.get("K_PSUM2", "0") == "1"  # fall back to 2 score bufs

_CACHE = {}


def _plan_qb(qb):
    """Chunks for one q-block: list of (g0, gw, na); na = tiles on ACT
    (leading), rest on DVE. With G=2 every chunk is homogeneous."""
    nkt = 2 * qb + 2
    if qb == 0:
        return nkt, [(0, 2, 2)]  # ACT; masked multiplicatively
    chunks = []
    for g0 in range(0, nkt, G):
        dve = (g0 == nkt - 2) or (g0 == nkt - 4 and qb in EXTRA_QBS)
        chunks.append((g0, G, 0 if dve else G))
    return nkt, chunks


def _order_chunks(chunks):
    """Interleave DVE chunks between ACT chunks so ACT never bubbles."""
    act = [c for c in chunks if c[2] > 0]
    dve = [c for c in chunks if c[2] == 0]
    order = act[:]
    for j, c in enumerate(dve):
        pos = min(1 + 2 * j, len(order))
        order.insert(pos, c)
    return order


def _build_nc():
    import concourse.bass as bass  # noqa: F401
    import concourse.mybir as mybir
    import concourse.tile as tile
    from concourse import bacc

    f32 = mybir.dt.float32
    f16 = mybir.dt.float16
    i32 = mybir.dt.int32
    EXP = mybir.ActivationFunctionType.Exp
    MUL = mybir.AluOpType.mult
    ADD = mybir.AluOpType.add

    nc = bacc.Bacc("TRN2", target_bir_lowering=False, debug=False, num_devices=NCORES)

    # kt is pre-scaled by SCALE on the host.
    qt_d = nc.dram_tensor("qt", [HPC, D, S], f16, kind="ExternalInput").ap()
    kt_d = nc.dram_tensor("kt", [HPC, D, S], f16, kind="ExternalInput").ap()
    # v packed on host as [HPC, 128, NKT*(D+1)]: partition-major tiles.
    v_d = nc.dram_tensor("v", [HPC, 128, NKT * (D + 1)], f16, kind="ExternalInput").ap()
    # Additive Schraudolph masks [128, 2*1024] f32: per head [Bf, Bf, mA, mB].
    madd_d = nc.dram_tensor("madd", [128, 2048], f32, kind="ExternalInput").ap()
    # Multiplicative fp16 masks for q-block 0: [mA, mB] x2 heads.
    m16_d = nc.dram_tensor("m16", [128, 1024], f16, kind="ExternalInput").ap()
    o_d = nc.dram_tensor("outT", [HPC, D + 1, S], f32, kind="ExternalOutput").ap()

    NSG = 2 if _PSUM2 else 3

    with tile.TileContext(nc) as tc:
        sb_pool = tc.alloc_tile_pool(name="sb", bufs=1)
        rot_pool = tc.alloc_tile_pool(name="rot", bufs=4)
        psum_pool = tc.alloc_tile_pool(name="psum", bufs=1, space="PSUM")

        # PSUM: allocate in tag order -> sG0 {0,1}, sG1 {2,3}, sG2 {4,5},
        # pv0 @6, pv1 @7.
        sg_tiles = {}
        for i in range(NSG):
            sg_tiles[i] = psum_pool.tile(
                [128, 2 * G * W], f32, tag=f"sG{i}", name=f"sG{i}"
            )
        if _PSUM2:
            _pad0 = psum_pool.tile([1, 512], f32, tag="pad0", name="pad0")
        pvt = {}
        pvt[0] = psum_pool.tile([D + 1, 2 * W], f32, tag="pv0", name="pv0")
        if _PSUM2:
            _pad1 = psum_pool.tile([1, 512], f32, tag="pad1", name="pad1")
        pvt[1] = psum_pool.tile([D + 1, 2 * W], f32, tag="pv1", name="pv1")
        _cnt = {"ps": 0, "pv": 0}

        def next_sg():
            i = _cnt["ps"] % NSG
            _cnt["ps"] += 1
            return psum_pool.tile(
                [128, 2 * G * W], f32, tag=f"sG{i}", name=f"sG{i}"
            )

        def next_pv():
            i = _cnt["pv"] % 2
            _cnt["pv"] += 1
            return psum_pool.tile(
                [D + 1, 2 * W], f32, tag=f"pv{i}", name=f"pv{i}"
            )

        madd = sb_pool.tile([128, 2048], f32, tag="madd")
        m16 = sb_pool.tile([128, 1024], f16, tag="m16")

        ktc = {}
        qtc = {}
        vxc = {}
        for pr in range(2):
            ktc[pr] = sb_pool.tile([128, S], f16, tag=f"ktc{pr}", name=f"ktc{pr}")
            qtc[pr] = sb_pool.tile([128, S], f16, tag=f"qtc{pr}", name=f"qtc{pr}")
        for h in range(HPC):
            vxc[h] = sb_pool.tile([128, NKT * (D + 1)], f16, tag=f"vx{h}",
                                  name=f"vx{h}")

        # ---- input DMA, criticality-ordered ----
        def ldkq(pr, dst, src, sl):
            hsl = slice(2 * pr, 2 * pr + 2)
            nc.sync.dma_start(
                dst[:, sl], src[hsl, :, sl].rearrange("h d s -> (h d) s")
            )

        ldkq(0, ktc[0], kt_d, slice(0, 256))
        ldkq(0, qtc[0], qt_d, slice(1792, 2048))
        ldkq(1, ktc[1], kt_d, slice(0, 256))
        ldkq(1, qtc[1], qt_d, slice(1792, 2048))
        nc.sync.dma_start(madd[:], madd_d)
        ldkq(0, ktc[0], kt_d, slice(256, 1024))
        ldkq(1, ktc[1], kt_d, slice(256, 1024))
        nc.sync.dma_start(vxc[0][:], v_d[0])
        nc.sync.dma_start(vxc[1][:], v_d[1])
        nc.sync.dma_start(vxc[2][:], v_d[2])
        nc.sync.dma_start(vxc[3][:], v_d[3])
        ldkq(0, ktc[0], kt_d, slice(1024, 2048))
        ldkq(1, ktc[1], kt_d, slice(1024, 2048))
        ldkq(0, qtc[0], qt_d, slice(1024, 1792))
        ldkq(1, qtc[1], qt_d, slice(1024, 1792))
        nc.sync.dma_start(m16[:], m16_d)
        ldkq(0, qtc[0], qt_d, slice(0, 1024))
        ldkq(1, qtc[1], qt_d, slice(0, 1024))

        def ktile(pr, kt):
            return ktc[pr][:, kt * TK:(kt + 1) * TK]

        def vx(h, kt):
            return vxc[h][:, kt * (D + 1):(kt + 1) * (D + 1)]

        pending = None  # (pr, qb, g0, gw, p, pv, first, last)

        def flush_pending():
            nonlocal pending
            if pending is None:
                return
            pr, qb, g0, gw, p, pv, first, last = pending
            hA, hB = 2 * pr, 2 * pr + 1
            nkt = 2 * qb + 2
            for j in range(gw):
                kt = g0 + j
                for head, vxt in ((0, vx(hA, kt)), (1, vx(hB, kt))):
                    nc.tensor.matmul(
                        pv[:, head * W:(head + 1) * W],
                        vxt,
                        p[:, head * gw * W + j * W:head * gw * W + (j + 1) * W],
                        start=(first and j == 0 and head == 0),
                        stop=(last and j == gw - 1),
                        skip_group_check=True,
                    )
            if last:  # write out the q-block
                o = rot_pool.tile([D + 1, 2 * W], f32, tag="o")
                nc.vector.tensor_copy(o[:], pv[:])
                nc.sync.dma_start(
                    o_d[hA:hA + 2, :, qb * W:(qb + 1) * W].rearrange(
                        "h d s -> d h s"
                    ),
                    o[:].rearrange("d (h s) -> d h s", h=2),
                )
            pending = None

        # One global stream: q-blocks descending, the two head-pairs
        # alternating, DVE chunks interleaved mid-q-block — no pipeline
        # drain at pair/q-block boundaries.
        for qb in reversed(range(NQB)):
            for pr in range(2):
                nkt, chunks = _plan_qb(qb)
                order = _order_chunks(chunks)
                pv = next_pv()
                qA = qtc[pr][0:64, qb * W:(qb + 1) * W]
                qB = qtc[pr][64:128, qb * W:(qb + 1) * W]
                for ci, (g0, gw, na) in enumerate(order):
                    sG = next_sg()
                    for j in range(gw):
                        kt = g0 + j
                        nc.tensor.matmul(
                            sG[:, j * W:(j + 1) * W],
                            ktile(pr, kt)[0:64], qA,
                            start=True, stop=True,
                        )
                        nc.tensor.matmul(
                            sG[:, gw * W + j * W:gw * W + (j + 1) * W],
                            ktile(pr, kt)[64:128], qB,
                            start=True, stop=True,
                        )
                    p = rot_pool.tile([128, 2 * G * W], f16, tag="p")
                    sGh = sG[:, :2 * gw * W].rearrange("q (h c) -> q h c", h=2)
                    ph = p[:, :2 * gw * W].rearrange("q (h c) -> q h c", h=2)
                    if na > 0:  # ACT chunk (na == gw)
                        nc.scalar.activation(
                            p[:, :2 * gw * W], sG[:, :2 * gw * W], EXP
                        )
                        if qb == 0:
                            nc.vector.tensor_mul(
                                p[:, :2 * gw * W], p[:, :2 * gw * W], m16[:]
                            )
                    else:  # DVE chunk: Schraudolph exp with fused mask
                        # mask col for tile kt: 512 + (kt - (nkt-2))*256
                        c0 = 512 + (g0 - (nkt - 2)) * W
                        t = rot_pool.tile([128, 2 * G * W], i32, tag="t")
                        th = t[:].rearrange("q (h c) -> q h c", h=2)
                        nc.vector.scalar_tensor_tensor(
                            th,
                            sGh,
                            float(A_SCH),
                            madd[:].rearrange("q (h c) -> q h c", h=2)[
                                :, :, c0:c0 + gw * W
                            ],
                            op0=MUL,
                            op1=ADD,
                        )
                        nc.vector.tensor_copy(ph, th.bitcast(f32))
                    flush_pending()
                    pending = (pr, qb, g0, gw, p, pv,
                               ci == 0, ci == len(order) - 1)
        flush_pending()

        psum_pool.release()
        rot_pool.release()
        sb_pool.release()

    nc.compile()
    return nc


def _get_nc():
    if "nc" not in _CACHE:
        _CACHE["nc"] = _build_nc()
    return _CACHE["nc"]


def _make_masks():
    p = np.arange(128)[:, None]
    c = np.arange(256)[None, :]
    mA = np.where(c >= p, B_SCH, GARB).astype(np.float32)
    mB = np.where(c >= p + 128, B_SCH, GARB).astype(np.float32)
    bf = np.full((128, 256), B_SCH, np.float32)
    head = np.concatenate([bf, bf, mA, mB], axis=1)  # [128, 1024]
    madd = np.concatenate([head, head], axis=1)      # [128, 2048]
    mA16 = (c >= p).astype(np.float16)
    mB16 = (c >= p + 128).astype(np.float16)
    h16 = np.concatenate([mA16, mB16], axis=1)       # [128, 512]
    m16 = np.concatenate([h16, h16], axis=1)         # [128, 1024]
    return madd, m16


def _prep_inputs(q, k, v):
    qf = np.ascontiguousarray(np.asarray(q, dtype=np.float32)).reshape(BH, S, D)
    kf = np.ascontiguousarray(np.asarray(k, dtype=np.float32)).reshape(BH, S, D)
    vf = np.ascontiguousarray(np.asarray(v, dtype=np.float32)).reshape(BH, S, D)
    vx = np.empty((BH, S, D + 1), np.float16)
    vx[:, :, :D] = vf
    vx[:, :, D] = 1.0
    vp = np.ascontiguousarray(
        vx.reshape(BH, NKT, 128, D + 1).transpose(0, 2, 1, 3)
    ).reshape(BH, 128, NKT * (D + 1))
    qt = qf.transpose(0, 2, 1).astype(np.float16)
    kt = (kf * SCALE).transpose(0, 2, 1).astype(np.float16)
    madd, m16 = _make_masks()
    in_maps = []
    for cid in range(NCORES):
        sl = slice(HPC * cid, HPC * (cid + 1))
        in_maps.append({
            "qt": np.ascontiguousarray(qt[sl]),
            "kt": np.ascontiguousarray(kt[sl]),
            "v": np.ascontiguousarray(vp[sl]),
            "madd": madd,
            "m16": m16,
        })
    return in_maps


def _postprocess(results):
    out = np.empty((B, H, S, D), np.float32)
    for cid in range(NCORES):
        ot = results[cid]["outT"]  # [HPC, D+1, S]
        o = (ot[:, :D, :] / ot[:, D:D + 1, :]).transpose(0, 2, 1)
        for i in range(HPC):
            bh = HPC * cid + i
            out[bh // H, bh % H] = o[i]
    return out


def run(q, k, v, trace=False):
    from concourse.bass_utils import run_bass_kernel_spmd

    nc = _get_nc()
    in_maps = _prep_inputs(q, k, v)
    res = run_bass_kernel_spmd(
        nc, in_maps, core_ids=list(range(NCORES)), trace=trace
    )
    return _postprocess(res.results), res


def kernel(q, k, v):
    out, _ = run(q, k, v, trace=False)
    return out
